# revision 29
# baseline (speedup 1.0000x reference)
"""Trainium2 Bass kernel for nn_AGREE (group-member attention + predict MLP).

Data-parallel across 8 NeuronCores: B=16384 samples sharded 2048/core,
embedding tables + MLP weights replicated.

Per sample b:
  mem_e = user_table[member_ids[b]]            [50, 64]
  item_e = item_table[item_inputs[b]]          [64]
  h = relu(concat(mem_e, item_e) @ att_w1+b1)  [50, 16]
  scores = h @ att_w2 (+b2, softmax-invariant) [50]
  at_wt = softmax(scores masked to m <= member_lengths[b])
  g = at_wt @ mem_e + group_table[group_inputs[b]]
  y = sigmoid(relu([g*item, g, item] @ pred_w1 + pred_b1) @ pred_w2 + pred_b2)

Layout strategy (v2 — bf16 X-bar transposes, N=512 batched matmuls):
  - indirect DMA gathers member rows with inline f32->bf16 cast
  - member-pair tiles transposed via HWDGE DMA-transpose (bf16) straight
    into [128, 512] SBUF batches spanning 4 sample-tiles; zero PE transposes
  - attention MLP: pair matmuls K=128 (2 members) -> PSUM, item part (+b1)
    fused via accumulate + ACT relu-with-bias; scores via block-diag w2
  - masked softmax batched over 4 tiles; weighted member sum via fused
    scalar_tensor_tensor chain (bf16 in, f32 accum); predict MLP N=512.
"""

import sys

sys.path.insert(0, "/opt/trn_rl_repo")

import numpy as np

from concourse import bacc, bass, mybir
from concourse.tile import TileContext

NC = 8
B, M, D = 16384, 50, 64
BL = B // NC  # samples per core
P = 128
NT = BL // P  # sample tiles per core (16)
ST = 4        # sample-tiles per super-tile
NS = NT // ST  # super-tiles (4)
SW = ST * P   # samples per super-tile (512)
HID = 16
G8 = 8        # members per score group
NGRP = (M + G8 - 1) // G8  # 7 (last group has 2 members)
F32 = mybir.dt.float32
BF16 = mybir.dt.bfloat16
I32 = mybir.dt.int32

NUM_USERS, NUM_ITEMS, NUM_GROUPS = 100000, 50000, 20000

AF = mybir.ActivationFunctionType
OP = mybir.AluOpType
AX = mybir.AxisListType

_CACHE = {}


def build_nc():
    nc = bacc.Bacc()

    # --- data inputs (per-core shards), host-arranged tile-major:
    # plane[p, t...] = value for sample t*128+p ---
    ids_ext = nc.declare_dram_parameter("m_ids", [P, NT * M], I32, isOutput=False)
    item_ext = nc.declare_dram_parameter("i_idx", [P, NT], I32, isOutput=False)
    grp_ext = nc.declare_dram_parameter("g_idx", [P, NT], I32, isOutput=False)
    len_ext = nc.declare_dram_parameter("m_len", [P, NT], F32, isOutput=False)
    user_ext = nc.declare_dram_parameter("user_t", [NUM_USERS, D], F32, isOutput=False)
    itab_ext = nc.declare_dram_parameter("item_t", [NUM_ITEMS, D], F32, isOutput=False)
    gtab_ext = nc.declare_dram_parameter("group_t", [NUM_GROUPS, D], F32, isOutput=False)

    # --- static weight rearrangements (host-prepared, bf16) ---
    w1u2_ext = nc.declare_dram_parameter("w1u2", [P, 2 * HID], BF16, isOutput=False)
    w1i4_ext = nc.declare_dram_parameter("w1i4", [D, 4 * HID], BF16, isOutput=False)
    w2blk_ext = nc.declare_dram_parameter("w2blk", [P, G8], BF16, isOutput=False)
    pweg_ext = nc.declare_dram_parameter("pw_eg", [2 * D, 8], BF16, isOutput=False)
    pwit_ext = nc.declare_dram_parameter("pw_it", [D, 8], BF16, isOutput=False)
    pw2_ext = nc.declare_dram_parameter("pw2", [8, 1], BF16, isOutput=False)
    b1r_ext = nc.declare_dram_parameter("b1r", [D, 1], F32, isOutput=False)
    ident_ext = nc.declare_dram_parameter("ident", [P, P], BF16, isOutput=False)
    pb1_ext = nc.declare_dram_parameter("pb1", [8, 1], F32, isOutput=False)

    out_ext = nc.declare_dram_parameter("out", [BL, 1], F32, isOutput=True)

    with TileContext(nc) as tc:
        with (
            tc.tile_pool(name="const", bufs=1) as cn,
            tc.tile_pool(name="gbf", bufs=6) as gp,
            tc.tile_pool(name="sbuf", bufs=3) as sb,
            tc.tile_pool(name="small", bufs=6) as sm,
            tc.tile_pool(name="bigT", bufs=2) as bt,
            tc.tile_pool(name="psA", bufs=2, space="PSUM") as psA,
            tc.tile_pool(name="psC", bufs=1, space="PSUM") as psC,
            tc.tile_pool(name="psP", bufs=1, space="PSUM") as psP,
            tc.tile_pool(name="psT", bufs=2, space="PSUM") as psT,
        ):
            # ---- constants ----
            w1u2 = cn.tile([P, 2 * HID], BF16)
            nc.sync.dma_start(out=w1u2[:], in_=w1u2_ext[:])
            w1i4 = cn.tile([D, 4 * HID], BF16)
            nc.sync.dma_start(out=w1i4[:], in_=w1i4_ext[:])
            w2blk = cn.tile([P, G8], BF16)
            nc.sync.dma_start(out=w2blk[:], in_=w2blk_ext[:])
            pweg = cn.tile([2 * D, 8], BF16)
            nc.sync.dma_start(out=pweg[:], in_=pweg_ext[:])
            pwit = cn.tile([D, 8], BF16)
            nc.sync.dma_start(out=pwit[:], in_=pwit_ext[:])
            pw2 = cn.tile([8, 1], BF16)
            nc.sync.dma_start(out=pw2[:], in_=pw2_ext[:])
            b1r = cn.tile([D, 1], F32)
            nc.sync.dma_start(out=b1r[:], in_=b1r_ext[:])
            pb1 = cn.tile([8, 1], F32)
            nc.sync.dma_start(out=pb1[:], in_=pb1_ext[:])
            ident = cn.tile([P, P], BF16)
            nc.sync.dma_start(out=ident[:], in_=ident_ext[:])
            ids_all = cn.tile([P, NT * M], I32)
            nc.sync.dma_start(out=ids_all[:], in_=ids_ext[:])
            iidx_all = cn.tile([P, NT], I32)
            nc.sync.dma_start(out=iidx_all[:], in_=item_ext[:])
            gidx_all = cn.tile([P, NT], I32)
            nc.sync.dma_start(out=gidx_all[:], in_=grp_ext[:])
            len_all = cn.tile([P, NT], F32)
            nc.sync.dma_start(out=len_all[:], in_=len_ext[:])
            # device iota over members (0..49), f32
            iota_i = cn.tile([P, M], I32)
            nc.gpsimd.iota(iota_i[:], pattern=[[1, M]], base=0, channel_multiplier=0)
            iota_m = cn.tile([P, M], F32)
            nc.vector.tensor_copy(out=iota_m[:], in_=iota_i[:])
            # absorb the len-plane DMA into the DVE clock once
            warm = cn.tile([P, 1], F32)
            nc.vector.tensor_copy(out=warm[:], in_=len_all[:, 0:1])

            for s in range(NS):
                # ---------- gathers (bf16 inline cast) ----------
                g_bf = []
                itgr = []
                gr_f = []
                for tp in range(ST):
                    t = s * ST + tp
                    gb = gp.tile([P, M * D], BF16, tag="gbf")
                    nc.gpsimd.indirect_dma_start(
                        out=gb[:], out_offset=None, in_=user_ext[:],
                        in_offset=bass.IndirectOffsetOnAxis(
                            ap=ids_all[:, t * M : (t + 1) * M], axis=0),
                    )
                    g_bf.append(gb)
                    ig = sm.tile([P, 2 * D], BF16, tag="itgr")
                    nc.gpsimd.indirect_dma_start(
                        out=ig[:, :D], out_offset=None, in_=itab_ext[:],
                        in_offset=bass.IndirectOffsetOnAxis(
                            ap=iidx_all[:, t : t + 1], axis=0),
                    )
                    nc.gpsimd.indirect_dma_start(
                        out=ig[:, D:], out_offset=None, in_=gtab_ext[:],
                        in_offset=bass.IndirectOffsetOnAxis(
                            ap=gidx_all[:, t : t + 1], axis=0),
                    )
                    itgr.append(ig)
                    gf = sm.tile([P, D], F32, tag="grf")
                    nc.gpsimd.indirect_dma_start(
                        out=gf[:], out_offset=None, in_=gtab_ext[:],
                        in_offset=bass.IndirectOffsetOnAxis(
                            ap=gidx_all[:, t : t + 1], axis=0),
                    )
                    gr_f.append(gf)

                # ---------- X-bar transposes (item/group) ----------
                # itT[:, 128*tp+...]: rows 0:64 = item_e^T for tile tp
                itT = bt.tile([P, SW], BF16, tag="itT")
                for tp in range(ST):
                    tps = psT.tile([P, P], BF16, tag="tps")
                    nc.tensor.transpose(out=tps[:], in_=itgr[tp][:], identity=ident[:])
                    eng = nc.vector if tp % 2 == 0 else nc.scalar
                    if tp % 2 == 0:
                        nc.vector.tensor_copy(out=itT[:, tp * P : (tp + 1) * P], in_=tps[:])
                    else:
                        nc.scalar.activation(out=itT[:, tp * P : (tp + 1) * P], in_=tps[:], func=AF.Copy)

                # ---------- attention MLP ----------
                scps = psC.tile([P, ST * M], F32, tag="scps")
                for g in range(NGRP):
                    mg = min(G8, M - g * G8)
                    npr = (mg + 1) // 2  # member pairs in this group
                    rows = mg * HID
                    rows_a = min(rows, D)
                    rows_b = rows - rows_a
                    hp_a = psA.tile([D, 512], F32, tag="hpa")
                    hp_b = None
                    if rows_b > 0:
                        hp_b = psA.tile([D, 512], F32, tag="hpb")
                    pairTs = []
                    for j in range(npr):
                        q = 4 * g + j
                        pairT = sb.tile([P, SW], BF16, tag="pairT")
                        # two [128, 256] psum stages -> two copies per pair
                        for half in range(2):
                            tps = psT.tile([P, 2 * P], BF16, tag="tps")
                            for k in range(2):
                                tp = 2 * half + k
                                nc.tensor.transpose(
                                    out=tps[:, k * P : (k + 1) * P],
                                    in_=g_bf[tp][:, 2 * q * D : 2 * (q + 1) * D],
                                    identity=ident[:],
                                )
                            if (j + half) % 2 == 0:
                                nc.vector.tensor_copy(
                                    out=pairT[:, half * 2 * P : (half + 1) * 2 * P],
                                    in_=tps[:])
                            else:
                                nc.scalar.activation(
                                    out=pairT[:, half * 2 * P : (half + 1) * 2 * P],
                                    in_=tps[:], func=AF.Copy)
                        pairTs.append(pairT)
                    for j in range(npr):
                        hp_t = hp_a if j < 2 else hp_b
                        off = 2 * HID * (j % 2)
                        nc.tensor.matmul(
                            out=hp_t[off : off + 2 * HID, :SW],
                            lhsT=w1u2[:], rhs=pairTs[j][:],
                            start=True, stop=False, skip_group_check=True,
                        )
                    # item part accumulates over the opened regions, closes
                    nc.tensor.matmul(
                        out=hp_a[:rows_a, :SW], lhsT=w1i4[:, :rows_a],
                        rhs=itT[:D, :], start=False, stop=True,
                        skip_group_check=True,
                    )
                    if rows_b > 0:
                        nc.tensor.matmul(
                            out=hp_b[:rows_b, :SW], lhsT=w1i4[:, :rows_b],
                            rhs=itT[:D, :], start=False, stop=True,
                            skip_group_check=True,
                        )
                    # relu(x + b1) on ACT, straight to bf16
                    ht4 = sb.tile([P, SW], BF16, tag="ht4")
                    nc.scalar.activation(
                        out=ht4[:rows_a, :], in_=hp_a[:rows_a, :SW],
                        func=AF.Relu, bias=b1r[:rows_a, :],
                    )
                    if rows_b > 0:
                        nc.scalar.activation(
                            out=ht4[D : D + rows_b, :], in_=hp_b[:rows_b, :SW],
                            func=AF.Relu, bias=b1r[:rows_b, :],
                        )
                    # scores for this group land [128 samples, mg] per tile
                    for tp in range(ST):
                        nc.tensor.matmul(
                            out=scps[:, tp * M + g * G8 : tp * M + g * G8 + mg],
                            lhsT=ht4[:rows, tp * P : (tp + 1) * P],
                            rhs=w2blk[:rows, :mg],
                            start=True, stop=True,
                        )

                # ---------- masked softmax (batched over 4 tiles) ----------
                msk = sb.tile([P, ST * M], F32, tag="msk")
                for tp in range(ST):
                    nc.vector.tensor_scalar(
                        out=msk[:, tp * M : (tp + 1) * M], in0=iota_m[:],
                        scalar1=len_all[:, s * ST + tp : s * ST + tp + 1],
                        scalar2=None, op0=OP.is_le,
                    )
                scm = sb.tile([P, ST * M], F32, tag="scm")
                nc.vector.scalar_tensor_tensor(
                    out=scm[:], in0=scps[:], scalar=30.0, in1=msk[:],
                    op0=OP.add, op1=OP.mult,
                )
                mx4 = sb.tile([P, ST], F32, tag="mx4")
                nc.vector.tensor_reduce(
                    out=mx4[:], in_=scm[:].rearrange("p (t m) -> p t m", m=M),
                    axis=AX.X, op=OP.max,
                )
                ein = sb.tile([P, ST * M], F32, tag="ein")
                nc.vector.tensor_tensor(
                    out=ein[:].rearrange("p (t m) -> p t m", m=M),
                    in0=scm[:].rearrange("p (t m) -> p t m", m=M),
                    in1=mx4[:].rearrange("p (t one) -> p t one", one=1)
                    .to_broadcast([P, ST, M]),
                    op=OP.subtract,
                )
                e4 = sb.tile([P, ST * M], F32, tag="e4")
                nc.scalar.activation(out=e4[:], in_=ein[:], func=AF.Exp)
                z4 = sb.tile([P, ST], F32, tag="z4")
                nc.vector.tensor_reduce(
                    out=z4[:], in_=e4[:].rearrange("p (t m) -> p t m", m=M),
                    axis=AX.X, op=OP.add,
                )
                rz4 = sb.tile([P, ST], F32, tag="rz4")
                nc.vector.reciprocal(out=rz4[:], in_=z4[:])


                # ---------- weighted member sum + g, per tile ----------
                e_bf = sb.tile([P, ST * M], BF16, tag="ebf")
                nc.vector.tensor_copy(out=e_bf[:], in_=e4[:])
                egT = bt.tile([P, SW], BF16, tag="egT")
                for tp in range(ST):
                    # weighted rows: all-bf16 product, then reduce over members
                    prod = sm.tile([P, M * D], BF16, tag="prod")
                    nc.gpsimd.tensor_tensor(
                        out=prod[:].rearrange("p (m d) -> p m d", d=D),
                        in0=g_bf[tp][:].rearrange("p (m d) -> p m d", d=D),
                        in1=e_bf[:, tp * M : (tp + 1) * M]
                        .rearrange("p (m one) -> p m one", one=1)
                        .to_broadcast([P, M, D]),
                        op=OP.mult,
                    )
                    # log-tree pairwise adds over member blocks (contiguous reads)
                    tre = sm.tile([P, 25 * D], F32, tag="tre")
                    nc.vector.tensor_tensor(
                        out=tre[:].rearrange("p (m d) -> p m d", d=D),
                        in0=prod[:].rearrange(
                            "p (m two d) -> p m two d", two=2, d=D)[:, :, 0, :],
                        in1=prod[:].rearrange(
                            "p (m two d) -> p m two d", two=2, d=D)[:, :, 1, :],
                        op=OP.add,
                    )
                    n = 25
                    buf = tre
                    while n > 1:
                        k = n // 2
                        odd = n - 2 * k
                        nxt_t = sm.tile([P, (k + odd) * D], F32, tag=f"tr{n}")
                        nc.vector.tensor_tensor(
                            out=nxt_t[:, : k * D].rearrange("p (m d) -> p m d", d=D),
                            in0=buf[:, : 2 * k * D].rearrange(
                                "p (m two d) -> p m two d", two=2, d=D)[:, :, 0, :],
                            in1=buf[:, : 2 * k * D].rearrange(
                                "p (m two d) -> p m two d", two=2, d=D)[:, :, 1, :],
                            op=OP.add,
                        )
                        if odd:
                            nc.vector.tensor_copy(
                                out=nxt_t[:, k * D :], in_=buf[:, 2 * k * D :])
                        buf = nxt_t
                        n = k + odd
                    acc = buf
                    # eg rows: [elem (0:64) | g (64:128)] bf16, then transpose
                    eg = sm.tile([P, 2 * D], BF16, tag="eg")
                    nc.vector.scalar_tensor_tensor(
                        out=eg[:, D:], in0=acc[:],
                        scalar=rz4[:, tp : tp + 1], in1=gr_f[tp][:],
                        op0=OP.mult, op1=OP.add,
                    )
                    nc.vector.tensor_tensor(
                        out=eg[:, :D], in0=eg[:, D:], in1=itgr[tp][:, :D],
                        op=OP.mult,
                    )
                    tps2 = psT.tile([P, P], BF16, tag="tps")
                    nc.tensor.transpose(out=tps2[:], in_=eg[:], identity=ident[:])
                    nc.vector.tensor_copy(
                        out=egT[:, tp * P : (tp + 1) * P], in_=tps2[:])

                # ---------- predict MLP (N=512) ----------
                pp = psP.tile([8, 512], F32, tag="pp")
                nc.tensor.matmul(
                    out=pp[:, :SW], lhsT=pweg[:], rhs=egT[:],
                    start=True, stop=False, skip_group_check=True,
                )
                nc.tensor.matmul(
                    out=pp[:, :SW], lhsT=pwit[:], rhs=itT[:D, :],
                    start=False, stop=True, skip_group_check=True,
                )
                ph = sb.tile([8, SW], BF16, tag="ph")
                nc.scalar.activation(
                    out=ph[:], in_=pp[:, :SW], func=AF.Relu, bias=pb1[:]
                )
                y_ps = psP.tile([1, SW], F32, tag="pp")
                nc.tensor.matmul(
                    out=y_ps[:], lhsT=pw2[:], rhs=ph[:], start=True, stop=True
                )
                y_sb = sb.tile([1, SW], F32, tag="ysb")
                nc.scalar.activation(out=y_sb[:], in_=y_ps[:], func=AF.Sigmoid)
                nc.sync.dma_start(
                    out=out_ext[s * SW : (s + 1) * SW, :], in_=y_sb[:]
                )

    nc.compile()
    return nc


def _statics(att_w1, att_b1, att_w2, pred_w1, pred_b1, pred_w2, pred_b2):
    import ml_dtypes

    bf = ml_dtypes.bfloat16
    # member-pair projection: blockdiag(W1u, W1u)
    w1u2 = np.zeros((P, 2 * HID), dtype=np.float32)
    w1u2[:D, :HID] = att_w1[:D, :]
    w1u2[D:, HID:] = att_w1[:D, :]
    # item-part weights tiled over 4 member blocks (no bias row; b1 via ACT)
    w1i4 = np.tile(att_w1[D:, :], (1, 4))
    # block-diagonal w2 for scores
    w2blk = np.zeros((P, G8), dtype=np.float32)
    for j in range(G8):
        w2blk[j * HID : (j + 1) * HID, j] = att_w2[:, 0]
    # predict: rows [elem (pred_w1[0:64]) ; g (pred_w1[64:128])]
    pw_eg = pred_w1[: 2 * D, :]
    pw_it = pred_w1[2 * D :, :]
    b1r = np.tile(att_b1, 4).reshape(D, 1).astype(np.float32)
    return dict(
        w1u2=w1u2.astype(bf), w1i4=w1i4.astype(bf), w2blk=w2blk.astype(bf),
        pw_eg=pw_eg.astype(bf), pw_it=pw_it.astype(bf),
        pw2=pred_w2.astype(bf), b1r=b1r, ident=np.eye(P, dtype=np.float32).astype(bf),
        pb1=pred_b1.reshape(8, 1).astype(np.float32),
    )


def make_in_maps(**inputs):
    st = _statics(
        np.asarray(inputs["att_w1"], np.float32),
        np.asarray(inputs["att_b1"], np.float32),
        np.asarray(inputs["att_w2"], np.float32),
        np.asarray(inputs["pred_w1"], np.float32),
        np.asarray(inputs["pred_b1"], np.float32),
        np.asarray(inputs["pred_w2"], np.float32),
        np.asarray(inputs["pred_b2"], np.float32),
    )

    def tile_major(x):
        # [BL(, k)] -> [P, NT(*k)]: column-block t holds samples t*128..t*128+127
        x = x.reshape(NT, P, -1)
        return np.ascontiguousarray(x.transpose(1, 0, 2).reshape(P, -1))

    m_ids = np.asarray(inputs["member_ids"], np.int32).reshape(NC, BL, M)
    i_idx = np.asarray(inputs["item_inputs"], np.int32).reshape(NC, BL)
    g_idx = np.asarray(inputs["group_inputs"], np.int32).reshape(NC, BL)
    m_len = np.asarray(inputs["member_lengths"], np.float32).reshape(NC, BL)
    user_t = np.ascontiguousarray(np.asarray(inputs["user_table"], np.float32))
    item_t = np.ascontiguousarray(np.asarray(inputs["item_table"], np.float32))
    group_t = np.ascontiguousarray(np.asarray(inputs["group_table"], np.float32))

    in_maps = []
    for c in range(NC):
        in_maps.append(
            {
                "m_ids": tile_major(m_ids[c]),
                "i_idx": tile_major(i_idx[c]),
                "g_idx": tile_major(g_idx[c]),
                "m_len": tile_major(m_len[c]),
                "user_t": user_t,
                "item_t": item_t,
                "group_t": group_t,
                "w1u2": st["w1u2"],
                "w1i4": st["w1i4"],
                "w2blk": st["w2blk"],
                "pw_eg": st["pw_eg"],
                "pw_it": st["pw_it"],
                "pw2": st["pw2"],
                "b1r": st["b1r"],
                "pb1": st["pb1"],
                "ident": st["ident"],
            }
        )
    return in_maps


def get_nc():
    if "nc" not in _CACHE:
        _CACHE["nc"] = build_nc()
    return _CACHE["nc"]


def kernel(**inputs):
    from concourse.bass_utils import run_bass_kernel_spmd

    nc = get_nc()
    in_maps = make_in_maps(**inputs)
    res = run_bass_kernel_spmd(nc, in_maps, core_ids=list(range(NC)))
    return np.concatenate([r["out"] for r in res.results], axis=0)


# revision 30
# speedup vs baseline: 1.1173x; 1.1173x over previous
"""Trainium2 Bass kernel for nn_AGREE (group-member attention + predict MLP).

Data-parallel across 8 NeuronCores: B=16384 samples sharded 2048/core,
embedding tables + MLP weights replicated.

Per sample b:
  mem_e = user_table[member_ids[b]]            [50, 64]
  item_e = item_table[item_inputs[b]]          [64]
  h = relu(concat(mem_e, item_e) @ att_w1+b1)  [50, 16]
  scores = h @ att_w2 (+b2, softmax-invariant) [50]
  at_wt = softmax(scores masked to m <= member_lengths[b])
  g = at_wt @ mem_e + group_table[group_inputs[b]]
  y = sigmoid(relu([g*item, g, item] @ pred_w1 + pred_b1) @ pred_w2 + pred_b2)

Layout strategy (v2 — bf16 X-bar transposes, N=512 batched matmuls):
  - indirect DMA gathers member rows with inline f32->bf16 cast
  - member-pair tiles transposed via HWDGE DMA-transpose (bf16) straight
    into [128, 512] SBUF batches spanning 4 sample-tiles; zero PE transposes
  - attention MLP: pair matmuls K=128 (2 members) -> PSUM, item part (+b1)
    fused via accumulate + ACT relu-with-bias; scores via block-diag w2
  - masked softmax batched over 4 tiles; weighted member sum via fused
    scalar_tensor_tensor chain (bf16 in, f32 accum); predict MLP N=512.
"""

import sys

sys.path.insert(0, "/opt/trn_rl_repo")

import numpy as np

from concourse import bacc, bass, mybir
from concourse.tile import TileContext

NC = 8
B, M, D = 16384, 50, 64
BL = B // NC  # samples per core
P = 128
NT = BL // P  # sample tiles per core (16)
ST = 4        # sample-tiles per super-tile
NS = NT // ST  # super-tiles (4)
SW = ST * P   # samples per super-tile (512)
HID = 16
G8 = 8        # members per score group
NGRP = (M + G8 - 1) // G8  # 7 (last group has 2 members)
F32 = mybir.dt.float32
BF16 = mybir.dt.bfloat16
I32 = mybir.dt.int32

NUM_USERS, NUM_ITEMS, NUM_GROUPS = 100000, 50000, 20000

AF = mybir.ActivationFunctionType
OP = mybir.AluOpType
AX = mybir.AxisListType

_CACHE = {}


def build_nc():
    nc = bacc.Bacc()

    # --- data inputs (per-core shards), host-arranged tile-major:
    # plane[p, t...] = value for sample t*128+p ---
    ids_ext = nc.declare_dram_parameter("m_ids", [P, NT * M], I32, isOutput=False)
    item_ext = nc.declare_dram_parameter("i_idx", [P, NT], I32, isOutput=False)
    grp_ext = nc.declare_dram_parameter("g_idx", [P, NT], I32, isOutput=False)
    len_ext = nc.declare_dram_parameter("m_len", [P, NT], F32, isOutput=False)
    user_ext = nc.declare_dram_parameter("user_t", [NUM_USERS, D], F32, isOutput=False)
    itab_ext = nc.declare_dram_parameter("item_t", [NUM_ITEMS, D], F32, isOutput=False)
    gtab_ext = nc.declare_dram_parameter("group_t", [NUM_GROUPS, D], F32, isOutput=False)

    # --- static weight rearrangements (host-prepared, bf16) ---
    w1u2_ext = nc.declare_dram_parameter("w1u2", [P, 2 * HID], BF16, isOutput=False)
    w1i4_ext = nc.declare_dram_parameter("w1i4", [D, 4 * HID], BF16, isOutput=False)
    w2blk_ext = nc.declare_dram_parameter("w2blk", [P, G8], BF16, isOutput=False)
    pweg_ext = nc.declare_dram_parameter("pw_eg", [2 * D, 8], BF16, isOutput=False)
    pwit_ext = nc.declare_dram_parameter("pw_it", [D, 8], BF16, isOutput=False)
    pw2_ext = nc.declare_dram_parameter("pw2", [8, 1], BF16, isOutput=False)
    b1r_ext = nc.declare_dram_parameter("b1r", [D, 1], F32, isOutput=False)
    ident_ext = nc.declare_dram_parameter("ident", [P, P], BF16, isOutput=False)
    pb1_ext = nc.declare_dram_parameter("pb1", [8, 1], F32, isOutput=False)

    out_ext = nc.declare_dram_parameter("out", [BL, 1], F32, isOutput=True)

    with TileContext(nc) as tc:
        with (
            tc.tile_pool(name="const", bufs=1) as cn,
            tc.tile_pool(name="gbf", bufs=6) as gp,
            tc.tile_pool(name="sbuf", bufs=3) as sb,
            tc.tile_pool(name="small", bufs=6) as sm,
            tc.tile_pool(name="bigT", bufs=2) as bt,
            tc.tile_pool(name="psA", bufs=2, space="PSUM") as psA,
            tc.tile_pool(name="psC", bufs=1, space="PSUM") as psC,
            tc.tile_pool(name="psP", bufs=1, space="PSUM") as psP,
            tc.tile_pool(name="psT", bufs=2, space="PSUM") as psT,
        ):
            # ---- constants ----
            w1u2 = cn.tile([P, 2 * HID], BF16)
            nc.sync.dma_start(out=w1u2[:], in_=w1u2_ext[:])
            w1i4 = cn.tile([D, 4 * HID], BF16)
            nc.sync.dma_start(out=w1i4[:], in_=w1i4_ext[:])
            w2blk = cn.tile([P, G8], BF16)
            nc.sync.dma_start(out=w2blk[:], in_=w2blk_ext[:])
            pweg = cn.tile([2 * D, 8], BF16)
            nc.sync.dma_start(out=pweg[:], in_=pweg_ext[:])
            pwit = cn.tile([D, 8], BF16)
            nc.sync.dma_start(out=pwit[:], in_=pwit_ext[:])
            pw2 = cn.tile([8, 1], BF16)
            nc.sync.dma_start(out=pw2[:], in_=pw2_ext[:])
            b1r = cn.tile([D, 1], F32)
            nc.sync.dma_start(out=b1r[:], in_=b1r_ext[:])
            pb1 = cn.tile([8, 1], F32)
            nc.sync.dma_start(out=pb1[:], in_=pb1_ext[:])
            ident = cn.tile([P, P], BF16)
            nc.sync.dma_start(out=ident[:], in_=ident_ext[:])
            ids_all = cn.tile([P, NT * M], I32)
            nc.sync.dma_start(out=ids_all[:], in_=ids_ext[:])
            iidx_all = cn.tile([P, NT], I32)
            nc.sync.dma_start(out=iidx_all[:], in_=item_ext[:])
            gidx_all = cn.tile([P, NT], I32)
            nc.sync.dma_start(out=gidx_all[:], in_=grp_ext[:])
            len_all = cn.tile([P, NT], F32)
            nc.sync.dma_start(out=len_all[:], in_=len_ext[:])
            # device iota over members (0..49), f32
            iota_i = cn.tile([P, M], I32)
            nc.gpsimd.iota(iota_i[:], pattern=[[1, M]], base=0, channel_multiplier=0)
            iota_m = cn.tile([P, M], F32)
            nc.vector.tensor_copy(out=iota_m[:], in_=iota_i[:])
            # absorb the len-plane DMA into the DVE clock once
            warm = cn.tile([P, 1], F32)
            nc.vector.tensor_copy(out=warm[:], in_=len_all[:, 0:1])

            for s in range(NS):
                # ---------- gathers (bf16 inline cast) ----------
                g_bf = []
                itgr = []
                gr_f = []
                for tp in range(ST):
                    t = s * ST + tp
                    gb = gp.tile([P, M * D], BF16, tag="gbf")
                    nc.gpsimd.indirect_dma_start(
                        out=gb[:], out_offset=None, in_=user_ext[:],
                        in_offset=bass.IndirectOffsetOnAxis(
                            ap=ids_all[:, t * M : (t + 1) * M], axis=0),
                    )
                    g_bf.append(gb)
                    ig = sm.tile([P, 2 * D], BF16, tag="itgr")
                    nc.gpsimd.indirect_dma_start(
                        out=ig[:, :D], out_offset=None, in_=itab_ext[:],
                        in_offset=bass.IndirectOffsetOnAxis(
                            ap=iidx_all[:, t : t + 1], axis=0),
                    )
                    nc.gpsimd.indirect_dma_start(
                        out=ig[:, D:], out_offset=None, in_=gtab_ext[:],
                        in_offset=bass.IndirectOffsetOnAxis(
                            ap=gidx_all[:, t : t + 1], axis=0),
                    )
                    itgr.append(ig)
                    gf = sm.tile([P, D], F32, tag="grf")
                    nc.gpsimd.indirect_dma_start(
                        out=gf[:], out_offset=None, in_=gtab_ext[:],
                        in_offset=bass.IndirectOffsetOnAxis(
                            ap=gidx_all[:, t : t + 1], axis=0),
                    )
                    gr_f.append(gf)

                # ---------- X-bar transposes (item/group) ----------
                # itT[:, 128*tp+...]: rows 0:64 = item_e^T for tile tp
                itT = bt.tile([P, SW], BF16, tag="itT")
                for tp in range(ST):
                    tps = psT.tile([P, P], BF16, tag="tps")
                    nc.tensor.transpose(out=tps[:], in_=itgr[tp][:], identity=ident[:])
                    eng = nc.vector if tp % 2 == 0 else nc.scalar
                    if tp % 2 == 0:
                        nc.vector.tensor_copy(out=itT[:, tp * P : (tp + 1) * P], in_=tps[:])
                    else:
                        nc.scalar.activation(out=itT[:, tp * P : (tp + 1) * P], in_=tps[:], func=AF.Copy)

                # ---------- attention MLP ----------
                scps = psC.tile([P, ST * M], F32, tag="scps")
                for g in range(NGRP):
                    mg = min(G8, M - g * G8)
                    npr = (mg + 1) // 2  # member pairs in this group
                    rows = mg * HID
                    rows_a = min(rows, D)
                    rows_b = rows - rows_a
                    hp_a = psA.tile([D, 512], F32, tag="hpa")
                    hp_b = None
                    if rows_b > 0:
                        hp_b = psA.tile([D, 512], F32, tag="hpb")
                    pairTs = []
                    for j in range(npr):
                        q = 4 * g + j
                        pairT = sb.tile([P, SW], BF16, tag="pairT")
                        # two [128, 256] psum stages -> two copies per pair
                        for half in range(2):
                            tps = psT.tile([P, 2 * P], BF16, tag="tps")
                            for k in range(2):
                                tp = 2 * half + k
                                nc.tensor.transpose(
                                    out=tps[:, k * P : (k + 1) * P],
                                    in_=g_bf[tp][:, 2 * q * D : 2 * (q + 1) * D],
                                    identity=ident[:],
                                )
                            if (j + half) % 2 == 0:
                                nc.vector.tensor_copy(
                                    out=pairT[:, half * 2 * P : (half + 1) * 2 * P],
                                    in_=tps[:])
                            else:
                                nc.scalar.activation(
                                    out=pairT[:, half * 2 * P : (half + 1) * 2 * P],
                                    in_=tps[:], func=AF.Copy)
                        pairTs.append(pairT)
                    for j in range(npr):
                        hp_t = hp_a if j < 2 else hp_b
                        off = 2 * HID * (j % 2)
                        nc.tensor.matmul(
                            out=hp_t[off : off + 2 * HID, :SW],
                            lhsT=w1u2[:], rhs=pairTs[j][:],
                            start=True, stop=False, skip_group_check=True,
                        )
                    # item part accumulates over the opened regions, closes
                    nc.tensor.matmul(
                        out=hp_a[:rows_a, :SW], lhsT=w1i4[:, :rows_a],
                        rhs=itT[:D, :], start=False, stop=True,
                        skip_group_check=True,
                    )
                    if rows_b > 0:
                        nc.tensor.matmul(
                            out=hp_b[:rows_b, :SW], lhsT=w1i4[:, :rows_b],
                            rhs=itT[:D, :], start=False, stop=True,
                            skip_group_check=True,
                        )
                    # relu(x + b1) on ACT, straight to bf16
                    ht4 = sb.tile([P, SW], BF16, tag="ht4")
                    nc.scalar.activation(
                        out=ht4[:rows_a, :], in_=hp_a[:rows_a, :SW],
                        func=AF.Relu, bias=b1r[:rows_a, :],
                    )
                    if rows_b > 0:
                        nc.scalar.activation(
                            out=ht4[D : D + rows_b, :], in_=hp_b[:rows_b, :SW],
                            func=AF.Relu, bias=b1r[:rows_b, :],
                        )
                    # scores for this group land [128 samples, mg] per tile
                    for tp in range(ST):
                        nc.tensor.matmul(
                            out=scps[:, tp * M + g * G8 : tp * M + g * G8 + mg],
                            lhsT=ht4[:rows, tp * P : (tp + 1) * P],
                            rhs=w2blk[:rows, :mg],
                            start=True, stop=True,
                        )

                # ---------- masked softmax (batched over 4 tiles) ----------
                msk = sb.tile([P, ST * M], F32, tag="msk")
                for tp in range(ST):
                    nc.vector.tensor_scalar(
                        out=msk[:, tp * M : (tp + 1) * M], in0=iota_m[:],
                        scalar1=len_all[:, s * ST + tp : s * ST + tp + 1],
                        scalar2=None, op0=OP.is_le,
                    )
                scm = sb.tile([P, ST * M], F32, tag="scm")
                nc.vector.scalar_tensor_tensor(
                    out=scm[:], in0=scps[:], scalar=30.0, in1=msk[:],
                    op0=OP.add, op1=OP.mult,
                )
                mx4 = sb.tile([P, ST], F32, tag="mx4")
                nc.vector.tensor_reduce(
                    out=mx4[:], in_=scm[:].rearrange("p (t m) -> p t m", m=M),
                    axis=AX.X, op=OP.max,
                )
                ein = sb.tile([P, ST * M], F32, tag="ein")
                nc.vector.tensor_tensor(
                    out=ein[:].rearrange("p (t m) -> p t m", m=M),
                    in0=scm[:].rearrange("p (t m) -> p t m", m=M),
                    in1=mx4[:].rearrange("p (t one) -> p t one", one=1)
                    .to_broadcast([P, ST, M]),
                    op=OP.subtract,
                )
                e4 = sb.tile([P, ST * M], F32, tag="e4")
                nc.scalar.activation(out=e4[:], in_=ein[:], func=AF.Exp)
                z4 = sb.tile([P, ST], F32, tag="z4")
                nc.vector.tensor_reduce(
                    out=z4[:], in_=e4[:].rearrange("p (t m) -> p t m", m=M),
                    axis=AX.X, op=OP.add,
                )
                rz4 = sb.tile([P, ST], F32, tag="rz4")
                nc.vector.reciprocal(out=rz4[:], in_=z4[:])


                # ---------- weighted member sum + g, per tile ----------
                e_bf = sb.tile([P, ST * M], BF16, tag="ebf")
                nc.vector.tensor_copy(out=e_bf[:], in_=e4[:])
                egT = bt.tile([P, SW], BF16, tag="egT")
                for tp in range(ST):
                    # weighted rows: all-bf16 product, then reduce over members
                    prod = sm.tile([P, M * D], BF16, tag="prod")
                    nc.vector.tensor_tensor(
                        out=prod[:].rearrange("p (m d) -> p m d", d=D),
                        in0=g_bf[tp][:].rearrange("p (m d) -> p m d", d=D),
                        in1=e_bf[:, tp * M : (tp + 1) * M]
                        .rearrange("p (m one) -> p m one", one=1)
                        .to_broadcast([P, M, D]),
                        op=OP.mult,
                    )
                    # log-tree pairwise adds over member blocks (contiguous reads)
                    tre = sm.tile([P, 25 * D], F32, tag="tre")
                    nc.vector.tensor_tensor(
                        out=tre[:].rearrange("p (m d) -> p m d", d=D),
                        in0=prod[:].rearrange(
                            "p (m two d) -> p m two d", two=2, d=D)[:, :, 0, :],
                        in1=prod[:].rearrange(
                            "p (m two d) -> p m two d", two=2, d=D)[:, :, 1, :],
                        op=OP.add,
                    )
                    n = 25
                    buf = tre
                    while n > 1:
                        k = n // 2
                        odd = n - 2 * k
                        nxt_t = sm.tile([P, (k + odd) * D], F32, tag=f"tr{n}")
                        nc.vector.tensor_tensor(
                            out=nxt_t[:, : k * D].rearrange("p (m d) -> p m d", d=D),
                            in0=buf[:, : 2 * k * D].rearrange(
                                "p (m two d) -> p m two d", two=2, d=D)[:, :, 0, :],
                            in1=buf[:, : 2 * k * D].rearrange(
                                "p (m two d) -> p m two d", two=2, d=D)[:, :, 1, :],
                            op=OP.add,
                        )
                        if odd:
                            nc.vector.tensor_copy(
                                out=nxt_t[:, k * D :], in_=buf[:, 2 * k * D :])
                        buf = nxt_t
                        n = k + odd
                    acc = buf
                    # eg rows: [elem (0:64) | g (64:128)] bf16, then transpose
                    eg = sm.tile([P, 2 * D], BF16, tag="eg")
                    nc.vector.scalar_tensor_tensor(
                        out=eg[:, D:], in0=acc[:],
                        scalar=rz4[:, tp : tp + 1], in1=gr_f[tp][:],
                        op0=OP.mult, op1=OP.add,
                    )
                    nc.vector.tensor_tensor(
                        out=eg[:, :D], in0=eg[:, D:], in1=itgr[tp][:, :D],
                        op=OP.mult,
                    )
                    tps2 = psT.tile([P, P], BF16, tag="tps")
                    nc.tensor.transpose(out=tps2[:], in_=eg[:], identity=ident[:])
                    nc.vector.tensor_copy(
                        out=egT[:, tp * P : (tp + 1) * P], in_=tps2[:])

                # ---------- predict MLP (N=512) ----------
                pp = psP.tile([8, 512], F32, tag="pp")
                nc.tensor.matmul(
                    out=pp[:, :SW], lhsT=pweg[:], rhs=egT[:],
                    start=True, stop=False, skip_group_check=True,
                )
                nc.tensor.matmul(
                    out=pp[:, :SW], lhsT=pwit[:], rhs=itT[:D, :],
                    start=False, stop=True, skip_group_check=True,
                )
                ph = sb.tile([8, SW], BF16, tag="ph")
                nc.scalar.activation(
                    out=ph[:], in_=pp[:, :SW], func=AF.Relu, bias=pb1[:]
                )
                y_ps = psP.tile([1, SW], F32, tag="pp")
                nc.tensor.matmul(
                    out=y_ps[:], lhsT=pw2[:], rhs=ph[:], start=True, stop=True
                )
                y_sb = sb.tile([1, SW], F32, tag="ysb")
                nc.scalar.activation(out=y_sb[:], in_=y_ps[:], func=AF.Sigmoid)
                nc.sync.dma_start(
                    out=out_ext[s * SW : (s + 1) * SW, :], in_=y_sb[:]
                )

    nc.compile()
    return nc


def _statics(att_w1, att_b1, att_w2, pred_w1, pred_b1, pred_w2, pred_b2):
    import ml_dtypes

    bf = ml_dtypes.bfloat16
    # member-pair projection: blockdiag(W1u, W1u)
    w1u2 = np.zeros((P, 2 * HID), dtype=np.float32)
    w1u2[:D, :HID] = att_w1[:D, :]
    w1u2[D:, HID:] = att_w1[:D, :]
    # item-part weights tiled over 4 member blocks (no bias row; b1 via ACT)
    w1i4 = np.tile(att_w1[D:, :], (1, 4))
    # block-diagonal w2 for scores
    w2blk = np.zeros((P, G8), dtype=np.float32)
    for j in range(G8):
        w2blk[j * HID : (j + 1) * HID, j] = att_w2[:, 0]
    # predict: rows [elem (pred_w1[0:64]) ; g (pred_w1[64:128])]
    pw_eg = pred_w1[: 2 * D, :]
    pw_it = pred_w1[2 * D :, :]
    b1r = np.tile(att_b1, 4).reshape(D, 1).astype(np.float32)
    return dict(
        w1u2=w1u2.astype(bf), w1i4=w1i4.astype(bf), w2blk=w2blk.astype(bf),
        pw_eg=pw_eg.astype(bf), pw_it=pw_it.astype(bf),
        pw2=pred_w2.astype(bf), b1r=b1r, ident=np.eye(P, dtype=np.float32).astype(bf),
        pb1=pred_b1.reshape(8, 1).astype(np.float32),
    )


def make_in_maps(**inputs):
    st = _statics(
        np.asarray(inputs["att_w1"], np.float32),
        np.asarray(inputs["att_b1"], np.float32),
        np.asarray(inputs["att_w2"], np.float32),
        np.asarray(inputs["pred_w1"], np.float32),
        np.asarray(inputs["pred_b1"], np.float32),
        np.asarray(inputs["pred_w2"], np.float32),
        np.asarray(inputs["pred_b2"], np.float32),
    )

    def tile_major(x):
        # [BL(, k)] -> [P, NT(*k)]: column-block t holds samples t*128..t*128+127
        x = x.reshape(NT, P, -1)
        return np.ascontiguousarray(x.transpose(1, 0, 2).reshape(P, -1))

    m_ids = np.asarray(inputs["member_ids"], np.int32).reshape(NC, BL, M)
    i_idx = np.asarray(inputs["item_inputs"], np.int32).reshape(NC, BL)
    g_idx = np.asarray(inputs["group_inputs"], np.int32).reshape(NC, BL)
    m_len = np.asarray(inputs["member_lengths"], np.float32).reshape(NC, BL)
    user_t = np.ascontiguousarray(np.asarray(inputs["user_table"], np.float32))
    item_t = np.ascontiguousarray(np.asarray(inputs["item_table"], np.float32))
    group_t = np.ascontiguousarray(np.asarray(inputs["group_table"], np.float32))

    in_maps = []
    for c in range(NC):
        in_maps.append(
            {
                "m_ids": tile_major(m_ids[c]),
                "i_idx": tile_major(i_idx[c]),
                "g_idx": tile_major(g_idx[c]),
                "m_len": tile_major(m_len[c]),
                "user_t": user_t,
                "item_t": item_t,
                "group_t": group_t,
                "w1u2": st["w1u2"],
                "w1i4": st["w1i4"],
                "w2blk": st["w2blk"],
                "pw_eg": st["pw_eg"],
                "pw_it": st["pw_it"],
                "pw2": st["pw2"],
                "b1r": st["b1r"],
                "pb1": st["pb1"],
                "ident": st["ident"],
            }
        )
    return in_maps


def get_nc():
    if "nc" not in _CACHE:
        _CACHE["nc"] = build_nc()
    return _CACHE["nc"]


def kernel(**inputs):
    from concourse.bass_utils import run_bass_kernel_spmd

    nc = get_nc()
    in_maps = make_in_maps(**inputs)
    res = run_bass_kernel_spmd(nc, in_maps, core_ids=list(range(NC)))
    return np.concatenate([r["out"] for r in res.results], axis=0)


# revision 31
# speedup vs baseline: 1.1177x; 1.0003x over previous
"""Trainium2 Bass kernel for nn_AGREE (group-member attention + predict MLP).

Data-parallel across 8 NeuronCores: B=16384 samples sharded 2048/core,
embedding tables + MLP weights replicated.

Per sample b:
  mem_e = user_table[member_ids[b]]            [50, 64]
  item_e = item_table[item_inputs[b]]          [64]
  h = relu(concat(mem_e, item_e) @ att_w1+b1)  [50, 16]
  scores = h @ att_w2 (+b2, softmax-invariant) [50]
  at_wt = softmax(scores masked to m <= member_lengths[b])
  g = at_wt @ mem_e + group_table[group_inputs[b]]
  y = sigmoid(relu([g*item, g, item] @ pred_w1 + pred_b1) @ pred_w2 + pred_b2)

Layout strategy (v2 — bf16 X-bar transposes, N=512 batched matmuls):
  - indirect DMA gathers member rows with inline f32->bf16 cast
  - member-pair tiles transposed via HWDGE DMA-transpose (bf16) straight
    into [128, 512] SBUF batches spanning 4 sample-tiles; zero PE transposes
  - attention MLP: pair matmuls K=128 (2 members) -> PSUM, item part (+b1)
    fused via accumulate + ACT relu-with-bias; scores via block-diag w2
  - masked softmax batched over 4 tiles; weighted member sum via fused
    scalar_tensor_tensor chain (bf16 in, f32 accum); predict MLP N=512.
"""

import sys

sys.path.insert(0, "/opt/trn_rl_repo")

import numpy as np

from concourse import bacc, bass, mybir
from concourse.tile import TileContext

NC = 8
B, M, D = 16384, 50, 64
BL = B // NC  # samples per core
P = 128
NT = BL // P  # sample tiles per core (16)
ST = 4        # sample-tiles per super-tile
NS = NT // ST  # super-tiles (4)
SW = ST * P   # samples per super-tile (512)
HID = 16
G8 = 8        # members per score group
NGRP = (M + G8 - 1) // G8  # 7 (last group has 2 members)
F32 = mybir.dt.float32
BF16 = mybir.dt.bfloat16
I32 = mybir.dt.int32

NUM_USERS, NUM_ITEMS, NUM_GROUPS = 100000, 50000, 20000

AF = mybir.ActivationFunctionType
OP = mybir.AluOpType
AX = mybir.AxisListType

_CACHE = {}


def build_nc():
    nc = bacc.Bacc()

    # --- data inputs (per-core shards), host-arranged tile-major:
    # plane[p, t...] = value for sample t*128+p ---
    ids_ext = nc.declare_dram_parameter("m_ids", [P, NT * M], I32, isOutput=False)
    item_ext = nc.declare_dram_parameter("i_idx", [P, NT], I32, isOutput=False)
    grp_ext = nc.declare_dram_parameter("g_idx", [P, NT], I32, isOutput=False)
    len_ext = nc.declare_dram_parameter("m_len", [P, NT], F32, isOutput=False)
    user_ext = nc.declare_dram_parameter("user_t", [NUM_USERS, D], F32, isOutput=False)
    itab_ext = nc.declare_dram_parameter("item_t", [NUM_ITEMS, D], F32, isOutput=False)
    gtab_ext = nc.declare_dram_parameter("group_t", [NUM_GROUPS, D], F32, isOutput=False)

    # --- static weight rearrangements (host-prepared, bf16) ---
    w1u2_ext = nc.declare_dram_parameter("w1u2", [P, 2 * HID], BF16, isOutput=False)
    w1i4_ext = nc.declare_dram_parameter("w1i4", [D, 4 * HID], BF16, isOutput=False)
    w2blk_ext = nc.declare_dram_parameter("w2blk", [P, G8], BF16, isOutput=False)
    pweg_ext = nc.declare_dram_parameter("pw_eg", [2 * D, 8], BF16, isOutput=False)
    pwit_ext = nc.declare_dram_parameter("pw_it", [D, 8], BF16, isOutput=False)
    pw2_ext = nc.declare_dram_parameter("pw2", [8, 1], BF16, isOutput=False)
    b1r_ext = nc.declare_dram_parameter("b1r", [D, 1], F32, isOutput=False)
    ident_ext = nc.declare_dram_parameter("ident", [P, P], BF16, isOutput=False)
    pb1_ext = nc.declare_dram_parameter("pb1", [8, 1], F32, isOutput=False)

    out_ext = nc.declare_dram_parameter("out", [BL, 1], F32, isOutput=True)

    with TileContext(nc) as tc:
        with (
            tc.tile_pool(name="const", bufs=1) as cn,
            tc.tile_pool(name="gbf", bufs=8) as gp,
            tc.tile_pool(name="sbuf", bufs=4) as sb,
            tc.tile_pool(name="small", bufs=6) as sm,
            tc.tile_pool(name="wide", bufs=2) as wd,
            tc.tile_pool(name="bigT", bufs=2) as bt,
            tc.tile_pool(name="psA", bufs=2, space="PSUM") as psA,
            tc.tile_pool(name="psC", bufs=1, space="PSUM") as psC,
            tc.tile_pool(name="psP", bufs=1, space="PSUM") as psP,
            tc.tile_pool(name="psT", bufs=2, space="PSUM") as psT,
        ):
            # ---- constants ----
            w1u2 = cn.tile([P, 2 * HID], BF16)
            nc.sync.dma_start(out=w1u2[:], in_=w1u2_ext[:])
            w1i4 = cn.tile([D, 4 * HID], BF16)
            nc.sync.dma_start(out=w1i4[:], in_=w1i4_ext[:])
            w2blk = cn.tile([P, G8], BF16)
            nc.sync.dma_start(out=w2blk[:], in_=w2blk_ext[:])
            pweg = cn.tile([2 * D, 8], BF16)
            nc.sync.dma_start(out=pweg[:], in_=pweg_ext[:])
            pwit = cn.tile([D, 8], BF16)
            nc.sync.dma_start(out=pwit[:], in_=pwit_ext[:])
            pw2 = cn.tile([8, 1], BF16)
            nc.sync.dma_start(out=pw2[:], in_=pw2_ext[:])
            b1r = cn.tile([D, 1], F32)
            nc.sync.dma_start(out=b1r[:], in_=b1r_ext[:])
            pb1 = cn.tile([8, 1], F32)
            nc.sync.dma_start(out=pb1[:], in_=pb1_ext[:])
            ident = cn.tile([P, P], BF16)
            nc.sync.dma_start(out=ident[:], in_=ident_ext[:])
            ids_all = cn.tile([P, NT * M], I32)
            nc.sync.dma_start(out=ids_all[:], in_=ids_ext[:])
            iidx_all = cn.tile([P, NT], I32)
            nc.sync.dma_start(out=iidx_all[:], in_=item_ext[:])
            gidx_all = cn.tile([P, NT], I32)
            nc.sync.dma_start(out=gidx_all[:], in_=grp_ext[:])
            len_all = cn.tile([P, NT], F32)
            nc.sync.dma_start(out=len_all[:], in_=len_ext[:])
            # device iota over members (0..49), f32
            iota_i = cn.tile([P, M], I32)
            nc.gpsimd.iota(iota_i[:], pattern=[[1, M]], base=0, channel_multiplier=0)
            iota_m = cn.tile([P, M], F32)
            nc.vector.tensor_copy(out=iota_m[:], in_=iota_i[:])
            # absorb the len-plane DMA into the DVE clock once
            warm = cn.tile([P, 1], F32)
            nc.vector.tensor_copy(out=warm[:], in_=len_all[:, 0:1])

            for s in range(NS):
                # ---------- gathers (bf16 inline cast) ----------
                g_bf = []
                itgr = []
                gr_f = []
                for tp in range(ST):
                    t = s * ST + tp
                    gb = gp.tile([P, M * D], BF16, tag="gbf")
                    nc.gpsimd.indirect_dma_start(
                        out=gb[:], out_offset=None, in_=user_ext[:],
                        in_offset=bass.IndirectOffsetOnAxis(
                            ap=ids_all[:, t * M : (t + 1) * M], axis=0),
                    )
                    g_bf.append(gb)
                    ig = sm.tile([P, 2 * D], BF16, tag="itgr")
                    nc.gpsimd.indirect_dma_start(
                        out=ig[:, :D], out_offset=None, in_=itab_ext[:],
                        in_offset=bass.IndirectOffsetOnAxis(
                            ap=iidx_all[:, t : t + 1], axis=0),
                    )
                    nc.gpsimd.indirect_dma_start(
                        out=ig[:, D:], out_offset=None, in_=gtab_ext[:],
                        in_offset=bass.IndirectOffsetOnAxis(
                            ap=gidx_all[:, t : t + 1], axis=0),
                    )
                    itgr.append(ig)
                    gf = sm.tile([P, D], F32, tag="grf")
                    nc.gpsimd.indirect_dma_start(
                        out=gf[:], out_offset=None, in_=gtab_ext[:],
                        in_offset=bass.IndirectOffsetOnAxis(
                            ap=gidx_all[:, t : t + 1], axis=0),
                    )
                    gr_f.append(gf)

                # ---------- X-bar transposes (item/group) ----------
                # itT[:, 128*tp+...]: rows 0:64 = item_e^T for tile tp
                itT = bt.tile([P, SW], BF16, tag="itT")
                for tp in range(ST):
                    tps = psT.tile([P, P], BF16, tag="tps")
                    nc.tensor.transpose(out=tps[:], in_=itgr[tp][:], identity=ident[:])
                    eng = nc.vector if tp % 2 == 0 else nc.scalar
                    if tp % 2 == 0:
                        nc.vector.tensor_copy(out=itT[:, tp * P : (tp + 1) * P], in_=tps[:])
                    else:
                        nc.scalar.activation(out=itT[:, tp * P : (tp + 1) * P], in_=tps[:], func=AF.Copy)

                # ---------- attention MLP ----------
                scps = psC.tile([P, ST * M], F32, tag="scps")
                for g in range(NGRP):
                    mg = min(G8, M - g * G8)
                    npr = (mg + 1) // 2  # member pairs in this group
                    rows = mg * HID
                    rows_a = min(rows, D)
                    rows_b = rows - rows_a
                    hp_a = psA.tile([D, 512], F32, tag="hpa")
                    hp_b = None
                    if rows_b > 0:
                        hp_b = psA.tile([D, 512], F32, tag="hpb")
                    pairTs = []
                    for j in range(npr):
                        q = 4 * g + j
                        pairT = sb.tile([P, SW], BF16, tag="pairT")
                        # two [128, 256] psum stages -> two copies per pair
                        for half in range(2):
                            tps = psT.tile([P, 2 * P], BF16, tag="tps")
                            for k in range(2):
                                tp = 2 * half + k
                                nc.tensor.transpose(
                                    out=tps[:, k * P : (k + 1) * P],
                                    in_=g_bf[tp][:, 2 * q * D : 2 * (q + 1) * D],
                                    identity=ident[:],
                                )
                            if (j + half) % 2 == 0:
                                nc.vector.tensor_copy(
                                    out=pairT[:, half * 2 * P : (half + 1) * 2 * P],
                                    in_=tps[:])
                            else:
                                nc.scalar.activation(
                                    out=pairT[:, half * 2 * P : (half + 1) * 2 * P],
                                    in_=tps[:], func=AF.Copy)
                        pairTs.append(pairT)
                    for j in range(npr):
                        hp_t = hp_a if j < 2 else hp_b
                        off = 2 * HID * (j % 2)
                        nc.tensor.matmul(
                            out=hp_t[off : off + 2 * HID, :SW],
                            lhsT=w1u2[:], rhs=pairTs[j][:],
                            start=True, stop=False, skip_group_check=True,
                        )
                    # item part accumulates over the opened regions, closes
                    nc.tensor.matmul(
                        out=hp_a[:rows_a, :SW], lhsT=w1i4[:, :rows_a],
                        rhs=itT[:D, :], start=False, stop=True,
                        skip_group_check=True,
                    )
                    if rows_b > 0:
                        nc.tensor.matmul(
                            out=hp_b[:rows_b, :SW], lhsT=w1i4[:, :rows_b],
                            rhs=itT[:D, :], start=False, stop=True,
                            skip_group_check=True,
                        )
                    # relu(x + b1) on ACT, straight to bf16
                    ht4 = sb.tile([P, SW], BF16, tag="ht4")
                    nc.scalar.activation(
                        out=ht4[:rows_a, :], in_=hp_a[:rows_a, :SW],
                        func=AF.Relu, bias=b1r[:rows_a, :],
                    )
                    if rows_b > 0:
                        nc.scalar.activation(
                            out=ht4[D : D + rows_b, :], in_=hp_b[:rows_b, :SW],
                            func=AF.Relu, bias=b1r[:rows_b, :],
                        )
                    # scores for this group land [128 samples, mg] per tile
                    for tp in range(ST):
                        nc.tensor.matmul(
                            out=scps[:, tp * M + g * G8 : tp * M + g * G8 + mg],
                            lhsT=ht4[:rows, tp * P : (tp + 1) * P],
                            rhs=w2blk[:rows, :mg],
                            start=True, stop=True,
                        )

                # ---------- masked softmax (batched over 4 tiles) ----------
                msk = sb.tile([P, ST * M], F32, tag="msk")
                for tp in range(ST):
                    nc.vector.tensor_scalar(
                        out=msk[:, tp * M : (tp + 1) * M], in0=iota_m[:],
                        scalar1=len_all[:, s * ST + tp : s * ST + tp + 1],
                        scalar2=None, op0=OP.is_le,
                    )
                scm = sb.tile([P, ST * M], F32, tag="scm")
                nc.vector.scalar_tensor_tensor(
                    out=scm[:], in0=scps[:], scalar=30.0, in1=msk[:],
                    op0=OP.add, op1=OP.mult,
                )
                mx4 = sb.tile([P, ST], F32, tag="mx4")
                nc.vector.tensor_reduce(
                    out=mx4[:], in_=scm[:].rearrange("p (t m) -> p t m", m=M),
                    axis=AX.X, op=OP.max,
                )
                ein = sb.tile([P, ST * M], F32, tag="ein")
                nc.vector.tensor_tensor(
                    out=ein[:].rearrange("p (t m) -> p t m", m=M),
                    in0=scm[:].rearrange("p (t m) -> p t m", m=M),
                    in1=mx4[:].rearrange("p (t one) -> p t one", one=1)
                    .to_broadcast([P, ST, M]),
                    op=OP.subtract,
                )
                e4 = sb.tile([P, ST * M], F32, tag="e4")
                nc.scalar.activation(out=e4[:], in_=ein[:], func=AF.Exp)
                z4 = sb.tile([P, ST], F32, tag="z4")
                nc.vector.tensor_reduce(
                    out=z4[:], in_=e4[:].rearrange("p (t m) -> p t m", m=M),
                    axis=AX.X, op=OP.add,
                )
                rz4 = sb.tile([P, ST], F32, tag="rz4")
                nc.vector.reciprocal(out=rz4[:], in_=z4[:])


                # ---------- weighted member sum + g, per tile ----------
                e_bf = sb.tile([P, ST * M], BF16, tag="ebf")
                nc.vector.tensor_copy(out=e_bf[:], in_=e4[:])
                egT = bt.tile([P, SW], BF16, tag="egT")
                for tp in range(ST):
                    # weighted rows: all-bf16 product, then reduce over members
                    prod = wd.tile([P, M * D], BF16, tag="prod")
                    nc.vector.tensor_tensor(
                        out=prod[:].rearrange("p (m d) -> p m d", d=D),
                        in0=g_bf[tp][:].rearrange("p (m d) -> p m d", d=D),
                        in1=e_bf[:, tp * M : (tp + 1) * M]
                        .rearrange("p (m one) -> p m one", one=1)
                        .to_broadcast([P, M, D]),
                        op=OP.mult,
                    )
                    # log-tree pairwise adds over member blocks (contiguous reads)
                    tre = wd.tile([P, 25 * D], F32, tag="tre")
                    nc.vector.tensor_tensor(
                        out=tre[:].rearrange("p (m d) -> p m d", d=D),
                        in0=prod[:].rearrange(
                            "p (m two d) -> p m two d", two=2, d=D)[:, :, 0, :],
                        in1=prod[:].rearrange(
                            "p (m two d) -> p m two d", two=2, d=D)[:, :, 1, :],
                        op=OP.add,
                    )
                    n = 25
                    buf = tre
                    while n > 1:
                        k = n // 2
                        odd = n - 2 * k
                        nxt_t = wd.tile([P, (k + odd) * D], F32, tag=f"tr{n}")
                        nc.vector.tensor_tensor(
                            out=nxt_t[:, : k * D].rearrange("p (m d) -> p m d", d=D),
                            in0=buf[:, : 2 * k * D].rearrange(
                                "p (m two d) -> p m two d", two=2, d=D)[:, :, 0, :],
                            in1=buf[:, : 2 * k * D].rearrange(
                                "p (m two d) -> p m two d", two=2, d=D)[:, :, 1, :],
                            op=OP.add,
                        )
                        if odd:
                            nc.vector.tensor_copy(
                                out=nxt_t[:, k * D :], in_=buf[:, 2 * k * D :])
                        buf = nxt_t
                        n = k + odd
                    acc = buf
                    # eg rows: [elem (0:64) | g (64:128)] bf16, then transpose
                    eg = sm.tile([P, 2 * D], BF16, tag="eg")
                    nc.vector.scalar_tensor_tensor(
                        out=eg[:, D:], in0=acc[:],
                        scalar=rz4[:, tp : tp + 1], in1=gr_f[tp][:],
                        op0=OP.mult, op1=OP.add,
                    )
                    nc.vector.tensor_tensor(
                        out=eg[:, :D], in0=eg[:, D:], in1=itgr[tp][:, :D],
                        op=OP.mult,
                    )
                    tps2 = psT.tile([P, P], BF16, tag="tps")
                    nc.tensor.transpose(out=tps2[:], in_=eg[:], identity=ident[:])
                    nc.vector.tensor_copy(
                        out=egT[:, tp * P : (tp + 1) * P], in_=tps2[:])

                # ---------- predict MLP (N=512) ----------
                pp = psP.tile([8, 512], F32, tag="pp")
                nc.tensor.matmul(
                    out=pp[:, :SW], lhsT=pweg[:], rhs=egT[:],
                    start=True, stop=False, skip_group_check=True,
                )
                nc.tensor.matmul(
                    out=pp[:, :SW], lhsT=pwit[:], rhs=itT[:D, :],
                    start=False, stop=True, skip_group_check=True,
                )
                ph = sb.tile([8, SW], BF16, tag="ph")
                nc.scalar.activation(
                    out=ph[:], in_=pp[:, :SW], func=AF.Relu, bias=pb1[:]
                )
                y_ps = psP.tile([1, SW], F32, tag="pp")
                nc.tensor.matmul(
                    out=y_ps[:], lhsT=pw2[:], rhs=ph[:], start=True, stop=True
                )
                y_sb = sb.tile([1, SW], F32, tag="ysb")
                nc.scalar.activation(out=y_sb[:], in_=y_ps[:], func=AF.Sigmoid)
                nc.sync.dma_start(
                    out=out_ext[s * SW : (s + 1) * SW, :], in_=y_sb[:]
                )

    nc.compile()
    return nc


def _statics(att_w1, att_b1, att_w2, pred_w1, pred_b1, pred_w2, pred_b2):
    import ml_dtypes

    bf = ml_dtypes.bfloat16
    # member-pair projection: blockdiag(W1u, W1u)
    w1u2 = np.zeros((P, 2 * HID), dtype=np.float32)
    w1u2[:D, :HID] = att_w1[:D, :]
    w1u2[D:, HID:] = att_w1[:D, :]
    # item-part weights tiled over 4 member blocks (no bias row; b1 via ACT)
    w1i4 = np.tile(att_w1[D:, :], (1, 4))
    # block-diagonal w2 for scores
    w2blk = np.zeros((P, G8), dtype=np.float32)
    for j in range(G8):
        w2blk[j * HID : (j + 1) * HID, j] = att_w2[:, 0]
    # predict: rows [elem (pred_w1[0:64]) ; g (pred_w1[64:128])]
    pw_eg = pred_w1[: 2 * D, :]
    pw_it = pred_w1[2 * D :, :]
    b1r = np.tile(att_b1, 4).reshape(D, 1).astype(np.float32)
    return dict(
        w1u2=w1u2.astype(bf), w1i4=w1i4.astype(bf), w2blk=w2blk.astype(bf),
        pw_eg=pw_eg.astype(bf), pw_it=pw_it.astype(bf),
        pw2=pred_w2.astype(bf), b1r=b1r, ident=np.eye(P, dtype=np.float32).astype(bf),
        pb1=pred_b1.reshape(8, 1).astype(np.float32),
    )


def make_in_maps(**inputs):
    st = _statics(
        np.asarray(inputs["att_w1"], np.float32),
        np.asarray(inputs["att_b1"], np.float32),
        np.asarray(inputs["att_w2"], np.float32),
        np.asarray(inputs["pred_w1"], np.float32),
        np.asarray(inputs["pred_b1"], np.float32),
        np.asarray(inputs["pred_w2"], np.float32),
        np.asarray(inputs["pred_b2"], np.float32),
    )

    def tile_major(x):
        # [BL(, k)] -> [P, NT(*k)]: column-block t holds samples t*128..t*128+127
        x = x.reshape(NT, P, -1)
        return np.ascontiguousarray(x.transpose(1, 0, 2).reshape(P, -1))

    m_ids = np.asarray(inputs["member_ids"], np.int32).reshape(NC, BL, M)
    i_idx = np.asarray(inputs["item_inputs"], np.int32).reshape(NC, BL)
    g_idx = np.asarray(inputs["group_inputs"], np.int32).reshape(NC, BL)
    m_len = np.asarray(inputs["member_lengths"], np.float32).reshape(NC, BL)
    user_t = np.ascontiguousarray(np.asarray(inputs["user_table"], np.float32))
    item_t = np.ascontiguousarray(np.asarray(inputs["item_table"], np.float32))
    group_t = np.ascontiguousarray(np.asarray(inputs["group_table"], np.float32))

    in_maps = []
    for c in range(NC):
        in_maps.append(
            {
                "m_ids": tile_major(m_ids[c]),
                "i_idx": tile_major(i_idx[c]),
                "g_idx": tile_major(g_idx[c]),
                "m_len": tile_major(m_len[c]),
                "user_t": user_t,
                "item_t": item_t,
                "group_t": group_t,
                "w1u2": st["w1u2"],
                "w1i4": st["w1i4"],
                "w2blk": st["w2blk"],
                "pw_eg": st["pw_eg"],
                "pw_it": st["pw_it"],
                "pw2": st["pw2"],
                "b1r": st["b1r"],
                "pb1": st["pb1"],
                "ident": st["ident"],
            }
        )
    return in_maps


def get_nc():
    if "nc" not in _CACHE:
        _CACHE["nc"] = build_nc()
    return _CACHE["nc"]


def kernel(**inputs):
    from concourse.bass_utils import run_bass_kernel_spmd

    nc = get_nc()
    in_maps = make_in_maps(**inputs)
    res = run_bass_kernel_spmd(nc, in_maps, core_ids=list(range(NC)))
    return np.concatenate([r["out"] for r in res.results], axis=0)


# revision 32
# speedup vs baseline: 1.3097x; 1.1718x over previous
"""Trainium2 Bass kernel for nn_AGREE (group-member attention + predict MLP).

Data-parallel across 8 NeuronCores: B=16384 samples sharded 2048/core,
embedding tables + MLP weights replicated.

Per sample b:
  mem_e = user_table[member_ids[b]]            [50, 64]
  item_e = item_table[item_inputs[b]]          [64]
  h = relu(concat(mem_e, item_e) @ att_w1+b1)  [50, 16]
  scores = h @ att_w2 (+b2, softmax-invariant) [50]
  at_wt = softmax(scores masked to m <= member_lengths[b])
  g = at_wt @ mem_e + group_table[group_inputs[b]]
  y = sigmoid(relu([g*item, g, item] @ pred_w1 + pred_b1) @ pred_w2 + pred_b2)

Layout strategy (v2 — bf16 X-bar transposes, N=512 batched matmuls):
  - indirect DMA gathers member rows with inline f32->bf16 cast
  - member-pair tiles transposed via HWDGE DMA-transpose (bf16) straight
    into [128, 512] SBUF batches spanning 4 sample-tiles; zero PE transposes
  - attention MLP: pair matmuls K=128 (2 members) -> PSUM, item part (+b1)
    fused via accumulate + ACT relu-with-bias; scores via block-diag w2
  - masked softmax batched over 4 tiles; weighted member sum via fused
    scalar_tensor_tensor chain (bf16 in, f32 accum); predict MLP N=512.
"""

import sys

sys.path.insert(0, "/opt/trn_rl_repo")

import numpy as np

from concourse import bacc, bass, mybir
from concourse.tile import TileContext

NC = 8
B, M, D = 16384, 50, 64
BL = B // NC  # samples per core
P = 128
NT = BL // P  # sample tiles per core (16)
ST = 4        # sample-tiles per super-tile
NS = NT // ST  # super-tiles (4)
SW = ST * P   # samples per super-tile (512)
HID = 16
G8 = 8        # members per score group
NGRP = (M + G8 - 1) // G8  # 7 (last group has 2 members)
F32 = mybir.dt.float32
BF16 = mybir.dt.bfloat16
I32 = mybir.dt.int32

NUM_USERS, NUM_ITEMS, NUM_GROUPS = 100000, 50000, 20000

AF = mybir.ActivationFunctionType
OP = mybir.AluOpType
AX = mybir.AxisListType

_CACHE = {}


def build_nc():
    nc = bacc.Bacc()

    # --- data inputs (per-core shards), host-arranged tile-major:
    # plane[p, t...] = value for sample t*128+p ---
    ids_ext = nc.declare_dram_parameter("m_ids", [P, NT * M], I32, isOutput=False)
    item_ext = nc.declare_dram_parameter("i_idx", [P, NT], I32, isOutput=False)
    grp_ext = nc.declare_dram_parameter("g_idx", [P, NT], I32, isOutput=False)
    len_ext = nc.declare_dram_parameter("m_len", [P, NT], F32, isOutput=False)
    user_ext = nc.declare_dram_parameter("user_t", [NUM_USERS, D], F32, isOutput=False)
    itab_ext = nc.declare_dram_parameter("item_t", [NUM_ITEMS, D], F32, isOutput=False)
    gtab_ext = nc.declare_dram_parameter("group_t", [NUM_GROUPS, D], F32, isOutput=False)

    # --- static weight rearrangements (host-prepared, bf16) ---
    w1u2_ext = nc.declare_dram_parameter("w1u2", [P, 2 * HID], BF16, isOutput=False)
    w1i4_ext = nc.declare_dram_parameter("w1i4", [D, 4 * HID], BF16, isOutput=False)
    w2blk_ext = nc.declare_dram_parameter("w2blk", [P, G8], BF16, isOutput=False)
    pweg_ext = nc.declare_dram_parameter("pw_eg", [2 * D, 8], BF16, isOutput=False)
    pwit_ext = nc.declare_dram_parameter("pw_it", [D, 8], BF16, isOutput=False)
    pw2_ext = nc.declare_dram_parameter("pw2", [8, 1], BF16, isOutput=False)
    b1r_ext = nc.declare_dram_parameter("b1r", [D, 1], F32, isOutput=False)
    ident_ext = nc.declare_dram_parameter("ident", [P, P], BF16, isOutput=False)
    pb1_ext = nc.declare_dram_parameter("pb1", [8, 1], F32, isOutput=False)

    out_ext = nc.declare_dram_parameter("out", [BL, 1], F32, isOutput=True)

    with TileContext(nc) as tc:
        with (
            tc.tile_pool(name="const", bufs=1) as cn,
            tc.tile_pool(name="gbf", bufs=8) as gp,
            tc.tile_pool(name="sbuf", bufs=4) as sb,
            tc.tile_pool(name="small", bufs=6) as sm,
            tc.tile_pool(name="wide", bufs=2) as wd,
            tc.tile_pool(name="bigT", bufs=2) as bt,
            tc.tile_pool(name="psA", bufs=2, space="PSUM") as psA,
            tc.tile_pool(name="psC", bufs=1, space="PSUM") as psC,
            tc.tile_pool(name="psP", bufs=1, space="PSUM") as psP,
            tc.tile_pool(name="psT", bufs=2, space="PSUM") as psT,
        ):
            # ---- constants ----
            w1u2 = cn.tile([P, 2 * HID], BF16)
            nc.sync.dma_start(out=w1u2[:], in_=w1u2_ext[:])
            w1i4 = cn.tile([D, 4 * HID], BF16)
            nc.sync.dma_start(out=w1i4[:], in_=w1i4_ext[:])
            w2blk = cn.tile([P, G8], BF16)
            nc.sync.dma_start(out=w2blk[:], in_=w2blk_ext[:])
            pweg = cn.tile([2 * D, 8], BF16)
            nc.sync.dma_start(out=pweg[:], in_=pweg_ext[:])
            pwit = cn.tile([D, 8], BF16)
            nc.sync.dma_start(out=pwit[:], in_=pwit_ext[:])
            pw2 = cn.tile([8, 1], BF16)
            nc.sync.dma_start(out=pw2[:], in_=pw2_ext[:])
            b1r = cn.tile([D, 1], F32)
            nc.sync.dma_start(out=b1r[:], in_=b1r_ext[:])
            pb1 = cn.tile([8, 1], F32)
            nc.sync.dma_start(out=pb1[:], in_=pb1_ext[:])
            ident = cn.tile([P, P], BF16)
            nc.sync.dma_start(out=ident[:], in_=ident_ext[:])
            ids_all = cn.tile([P, NT * M], I32)
            nc.sync.dma_start(out=ids_all[:], in_=ids_ext[:])
            iidx_all = cn.tile([P, NT], I32)
            nc.sync.dma_start(out=iidx_all[:], in_=item_ext[:])
            gidx_all = cn.tile([P, NT], I32)
            nc.sync.dma_start(out=gidx_all[:], in_=grp_ext[:])
            len_all = cn.tile([P, NT], F32)
            nc.sync.dma_start(out=len_all[:], in_=len_ext[:])
            # device iota over members (0..49), f32
            iota_i = cn.tile([P, M], I32)
            nc.gpsimd.iota(iota_i[:], pattern=[[1, M]], base=0, channel_multiplier=0)
            iota_m = cn.tile([P, M], F32)
            nc.vector.tensor_copy(out=iota_m[:], in_=iota_i[:])
            # absorb the len-plane DMA into the DVE clock once
            warm = cn.tile([P, 1], F32)
            nc.vector.tensor_copy(out=warm[:], in_=len_all[:, 0:1])

            def issue_gathers(s):
                # bf16 inline-cast gathers for one super-tile
                g_bf, itgr, gr_f = [], [], []
                for tp in range(ST):
                    t = s * ST + tp
                    gb = gp.tile([P, M * D], BF16, tag="gbf", name=f"gb{t}")
                    nc.gpsimd.indirect_dma_start(
                        out=gb[:], out_offset=None, in_=user_ext[:],
                        in_offset=bass.IndirectOffsetOnAxis(
                            ap=ids_all[:, t * M : (t + 1) * M], axis=0),
                    )
                    g_bf.append(gb)
                    ig = sm.tile([P, 2 * D], BF16, tag="itgr", name=f"ig{t}")
                    nc.gpsimd.indirect_dma_start(
                        out=ig[:, :D], out_offset=None, in_=itab_ext[:],
                        in_offset=bass.IndirectOffsetOnAxis(
                            ap=iidx_all[:, t : t + 1], axis=0),
                    )
                    nc.gpsimd.indirect_dma_start(
                        out=ig[:, D:], out_offset=None, in_=gtab_ext[:],
                        in_offset=bass.IndirectOffsetOnAxis(
                            ap=gidx_all[:, t : t + 1], axis=0),
                    )
                    itgr.append(ig)
                    gf = sm.tile([P, D], F32, tag="grf", name=f"gf{t}")
                    nc.gpsimd.indirect_dma_start(
                        out=gf[:], out_offset=None, in_=gtab_ext[:],
                        in_offset=bass.IndirectOffsetOnAxis(
                            ap=gidx_all[:, t : t + 1], axis=0),
                    )
                    gr_f.append(gf)
                return g_bf, itgr, gr_f

            pending = issue_gathers(0)
            for s in range(NS):
                g_bf, itgr, gr_f = pending
                # prefetch next super-tile's gathers before touching this one
                if s + 1 < NS:
                    pending = issue_gathers(s + 1)

                # ---------- X-bar transposes (item/group) ----------
                # itT[:, 128*tp+...]: rows 0:64 = item_e^T for tile tp
                itT = bt.tile([P, SW], BF16, tag="itT")
                for tp in range(ST):
                    tps = psT.tile([P, P], BF16, tag="tps")
                    nc.tensor.transpose(out=tps[:], in_=itgr[tp][:], identity=ident[:])
                    eng = nc.vector if tp % 2 == 0 else nc.scalar
                    if tp % 2 == 0:
                        nc.vector.tensor_copy(out=itT[:, tp * P : (tp + 1) * P], in_=tps[:])
                    else:
                        nc.scalar.activation(out=itT[:, tp * P : (tp + 1) * P], in_=tps[:], func=AF.Copy)

                # ---------- attention MLP ----------
                scps = psC.tile([P, ST * M], F32, tag="scps")
                for g in range(NGRP):
                    mg = min(G8, M - g * G8)
                    npr = (mg + 1) // 2  # member pairs in this group
                    rows = mg * HID
                    rows_a = min(rows, D)
                    rows_b = rows - rows_a
                    hp_a = psA.tile([D, 512], F32, tag="hpa")
                    hp_b = None
                    if rows_b > 0:
                        hp_b = psA.tile([D, 512], F32, tag="hpb")
                    pairTs = []
                    for j in range(npr):
                        q = 4 * g + j
                        pairT = sb.tile([P, SW], BF16, tag="pairT")
                        # two [128, 256] psum stages -> two copies per pair
                        for half in range(2):
                            tps = psT.tile([P, 2 * P], BF16, tag="tps")
                            for k in range(2):
                                tp = 2 * half + k
                                nc.tensor.transpose(
                                    out=tps[:, k * P : (k + 1) * P],
                                    in_=g_bf[tp][:, 2 * q * D : 2 * (q + 1) * D],
                                    identity=ident[:],
                                )
                            if (j + half) % 2 == 0:
                                nc.vector.tensor_copy(
                                    out=pairT[:, half * 2 * P : (half + 1) * 2 * P],
                                    in_=tps[:])
                            else:
                                nc.scalar.activation(
                                    out=pairT[:, half * 2 * P : (half + 1) * 2 * P],
                                    in_=tps[:], func=AF.Copy)
                        pairTs.append(pairT)
                    for j in range(npr):
                        hp_t = hp_a if j < 2 else hp_b
                        off = 2 * HID * (j % 2)
                        nc.tensor.matmul(
                            out=hp_t[off : off + 2 * HID, :SW],
                            lhsT=w1u2[:], rhs=pairTs[j][:],
                            start=True, stop=False, skip_group_check=True,
                        )
                    # item part accumulates over the opened regions, closes
                    nc.tensor.matmul(
                        out=hp_a[:rows_a, :SW], lhsT=w1i4[:, :rows_a],
                        rhs=itT[:D, :], start=False, stop=True,
                        skip_group_check=True,
                    )
                    if rows_b > 0:
                        nc.tensor.matmul(
                            out=hp_b[:rows_b, :SW], lhsT=w1i4[:, :rows_b],
                            rhs=itT[:D, :], start=False, stop=True,
                            skip_group_check=True,
                        )
                    # relu(x + b1) on ACT, straight to bf16
                    ht4 = sb.tile([P, SW], BF16, tag="ht4")
                    nc.scalar.activation(
                        out=ht4[:rows_a, :], in_=hp_a[:rows_a, :SW],
                        func=AF.Relu, bias=b1r[:rows_a, :],
                    )
                    if rows_b > 0:
                        nc.scalar.activation(
                            out=ht4[D : D + rows_b, :], in_=hp_b[:rows_b, :SW],
                            func=AF.Relu, bias=b1r[:rows_b, :],
                        )
                    # scores for this group land [128 samples, mg] per tile
                    for tp in range(ST):
                        nc.tensor.matmul(
                            out=scps[:, tp * M + g * G8 : tp * M + g * G8 + mg],
                            lhsT=ht4[:rows, tp * P : (tp + 1) * P],
                            rhs=w2blk[:rows, :mg],
                            start=True, stop=True,
                        )

                # ---------- masked softmax (batched over 4 tiles) ----------
                msk = sb.tile([P, ST * M], F32, tag="msk")
                for tp in range(ST):
                    nc.vector.tensor_scalar(
                        out=msk[:, tp * M : (tp + 1) * M], in0=iota_m[:],
                        scalar1=len_all[:, s * ST + tp : s * ST + tp + 1],
                        scalar2=None, op0=OP.is_le,
                    )
                scm = sb.tile([P, ST * M], F32, tag="scm")
                nc.vector.scalar_tensor_tensor(
                    out=scm[:], in0=scps[:], scalar=30.0, in1=msk[:],
                    op0=OP.add, op1=OP.mult,
                )
                mx4 = sb.tile([P, ST], F32, tag="mx4")
                nc.vector.tensor_reduce(
                    out=mx4[:], in_=scm[:].rearrange("p (t m) -> p t m", m=M),
                    axis=AX.X, op=OP.max,
                )
                ein = sb.tile([P, ST * M], F32, tag="ein")
                nc.vector.tensor_tensor(
                    out=ein[:].rearrange("p (t m) -> p t m", m=M),
                    in0=scm[:].rearrange("p (t m) -> p t m", m=M),
                    in1=mx4[:].rearrange("p (t one) -> p t one", one=1)
                    .to_broadcast([P, ST, M]),
                    op=OP.subtract,
                )
                e4 = sb.tile([P, ST * M], F32, tag="e4")
                nc.scalar.activation(out=e4[:], in_=ein[:], func=AF.Exp)
                z4 = sb.tile([P, ST], F32, tag="z4")
                nc.vector.tensor_reduce(
                    out=z4[:], in_=e4[:].rearrange("p (t m) -> p t m", m=M),
                    axis=AX.X, op=OP.add,
                )
                rz4 = sb.tile([P, ST], F32, tag="rz4")
                nc.vector.reciprocal(out=rz4[:], in_=z4[:])


                # ---------- weighted member sum + g, per tile ----------
                e_bf = sb.tile([P, ST * M], BF16, tag="ebf")
                nc.vector.tensor_copy(out=e_bf[:], in_=e4[:])
                egT = bt.tile([P, SW], BF16, tag="egT")
                for tp in range(ST):
                    # weighted rows: all-bf16 product, then reduce over members
                    prod = wd.tile([P, M * D], BF16, tag="prod")
                    nc.vector.tensor_tensor(
                        out=prod[:].rearrange("p (m d) -> p m d", d=D),
                        in0=g_bf[tp][:].rearrange("p (m d) -> p m d", d=D),
                        in1=e_bf[:, tp * M : (tp + 1) * M]
                        .rearrange("p (m one) -> p m one", one=1)
                        .to_broadcast([P, M, D]),
                        op=OP.mult,
                    )
                    # log-tree pairwise adds over member blocks (contiguous reads)
                    tre = wd.tile([P, 25 * D], F32, tag="tre")
                    nc.vector.tensor_tensor(
                        out=tre[:].rearrange("p (m d) -> p m d", d=D),
                        in0=prod[:].rearrange(
                            "p (m two d) -> p m two d", two=2, d=D)[:, :, 0, :],
                        in1=prod[:].rearrange(
                            "p (m two d) -> p m two d", two=2, d=D)[:, :, 1, :],
                        op=OP.add,
                    )
                    n = 25
                    buf = tre
                    while n > 1:
                        k = n // 2
                        odd = n - 2 * k
                        nxt_t = wd.tile([P, (k + odd) * D], F32, tag=f"tr{n}")
                        nc.vector.tensor_tensor(
                            out=nxt_t[:, : k * D].rearrange("p (m d) -> p m d", d=D),
                            in0=buf[:, : 2 * k * D].rearrange(
                                "p (m two d) -> p m two d", two=2, d=D)[:, :, 0, :],
                            in1=buf[:, : 2 * k * D].rearrange(
                                "p (m two d) -> p m two d", two=2, d=D)[:, :, 1, :],
                            op=OP.add,
                        )
                        if odd:
                            nc.vector.tensor_copy(
                                out=nxt_t[:, k * D :], in_=buf[:, 2 * k * D :])
                        buf = nxt_t
                        n = k + odd
                    acc = buf
                    # eg rows: [elem (0:64) | g (64:128)] bf16, then transpose
                    eg = sm.tile([P, 2 * D], BF16, tag="eg")
                    nc.vector.scalar_tensor_tensor(
                        out=eg[:, D:], in0=acc[:],
                        scalar=rz4[:, tp : tp + 1], in1=gr_f[tp][:],
                        op0=OP.mult, op1=OP.add,
                    )
                    nc.vector.tensor_tensor(
                        out=eg[:, :D], in0=eg[:, D:], in1=itgr[tp][:, :D],
                        op=OP.mult,
                    )
                    tps2 = psT.tile([P, P], BF16, tag="tps")
                    nc.tensor.transpose(out=tps2[:], in_=eg[:], identity=ident[:])
                    nc.vector.tensor_copy(
                        out=egT[:, tp * P : (tp + 1) * P], in_=tps2[:])

                # ---------- predict MLP (N=512) ----------
                pp = psP.tile([8, 512], F32, tag="pp")
                nc.tensor.matmul(
                    out=pp[:, :SW], lhsT=pweg[:], rhs=egT[:],
                    start=True, stop=False, skip_group_check=True,
                )
                nc.tensor.matmul(
                    out=pp[:, :SW], lhsT=pwit[:], rhs=itT[:D, :],
                    start=False, stop=True, skip_group_check=True,
                )
                ph = sb.tile([8, SW], BF16, tag="ph")
                nc.scalar.activation(
                    out=ph[:], in_=pp[:, :SW], func=AF.Relu, bias=pb1[:]
                )
                y_ps = psP.tile([1, SW], F32, tag="pp")
                nc.tensor.matmul(
                    out=y_ps[:], lhsT=pw2[:], rhs=ph[:], start=True, stop=True
                )
                y_sb = sb.tile([1, SW], F32, tag="ysb")
                nc.scalar.activation(out=y_sb[:], in_=y_ps[:], func=AF.Sigmoid)
                nc.sync.dma_start(
                    out=out_ext[s * SW : (s + 1) * SW, :], in_=y_sb[:]
                )

    nc.compile()
    return nc


def _statics(att_w1, att_b1, att_w2, pred_w1, pred_b1, pred_w2, pred_b2):
    import ml_dtypes

    bf = ml_dtypes.bfloat16
    # member-pair projection: blockdiag(W1u, W1u)
    w1u2 = np.zeros((P, 2 * HID), dtype=np.float32)
    w1u2[:D, :HID] = att_w1[:D, :]
    w1u2[D:, HID:] = att_w1[:D, :]
    # item-part weights tiled over 4 member blocks (no bias row; b1 via ACT)
    w1i4 = np.tile(att_w1[D:, :], (1, 4))
    # block-diagonal w2 for scores
    w2blk = np.zeros((P, G8), dtype=np.float32)
    for j in range(G8):
        w2blk[j * HID : (j + 1) * HID, j] = att_w2[:, 0]
    # predict: rows [elem (pred_w1[0:64]) ; g (pred_w1[64:128])]
    pw_eg = pred_w1[: 2 * D, :]
    pw_it = pred_w1[2 * D :, :]
    b1r = np.tile(att_b1, 4).reshape(D, 1).astype(np.float32)
    return dict(
        w1u2=w1u2.astype(bf), w1i4=w1i4.astype(bf), w2blk=w2blk.astype(bf),
        pw_eg=pw_eg.astype(bf), pw_it=pw_it.astype(bf),
        pw2=pred_w2.astype(bf), b1r=b1r, ident=np.eye(P, dtype=np.float32).astype(bf),
        pb1=pred_b1.reshape(8, 1).astype(np.float32),
    )


def make_in_maps(**inputs):
    st = _statics(
        np.asarray(inputs["att_w1"], np.float32),
        np.asarray(inputs["att_b1"], np.float32),
        np.asarray(inputs["att_w2"], np.float32),
        np.asarray(inputs["pred_w1"], np.float32),
        np.asarray(inputs["pred_b1"], np.float32),
        np.asarray(inputs["pred_w2"], np.float32),
        np.asarray(inputs["pred_b2"], np.float32),
    )

    def tile_major(x):
        # [BL(, k)] -> [P, NT(*k)]: column-block t holds samples t*128..t*128+127
        x = x.reshape(NT, P, -1)
        return np.ascontiguousarray(x.transpose(1, 0, 2).reshape(P, -1))

    m_ids = np.asarray(inputs["member_ids"], np.int32).reshape(NC, BL, M)
    i_idx = np.asarray(inputs["item_inputs"], np.int32).reshape(NC, BL)
    g_idx = np.asarray(inputs["group_inputs"], np.int32).reshape(NC, BL)
    m_len = np.asarray(inputs["member_lengths"], np.float32).reshape(NC, BL)
    user_t = np.ascontiguousarray(np.asarray(inputs["user_table"], np.float32))
    item_t = np.ascontiguousarray(np.asarray(inputs["item_table"], np.float32))
    group_t = np.ascontiguousarray(np.asarray(inputs["group_table"], np.float32))

    in_maps = []
    for c in range(NC):
        in_maps.append(
            {
                "m_ids": tile_major(m_ids[c]),
                "i_idx": tile_major(i_idx[c]),
                "g_idx": tile_major(g_idx[c]),
                "m_len": tile_major(m_len[c]),
                "user_t": user_t,
                "item_t": item_t,
                "group_t": group_t,
                "w1u2": st["w1u2"],
                "w1i4": st["w1i4"],
                "w2blk": st["w2blk"],
                "pw_eg": st["pw_eg"],
                "pw_it": st["pw_it"],
                "pw2": st["pw2"],
                "b1r": st["b1r"],
                "pb1": st["pb1"],
                "ident": st["ident"],
            }
        )
    return in_maps


def get_nc():
    if "nc" not in _CACHE:
        _CACHE["nc"] = build_nc()
    return _CACHE["nc"]


def kernel(**inputs):
    from concourse.bass_utils import run_bass_kernel_spmd

    nc = get_nc()
    in_maps = make_in_maps(**inputs)
    res = run_bass_kernel_spmd(nc, in_maps, core_ids=list(range(NC)))
    return np.concatenate([r["out"] for r in res.results], axis=0)
